# revision 16
# baseline (speedup 1.0000x reference)
"""MultiHeadSimilarity kernel for 8 Trainium2 NeuronCores.

Reference computation (per batch b):
    Q = wq @ x[b];  K = wk @ y[b]                       (channel-mixing matmuls)
    per head h (d=64):  A = relu(Qh^T Kh) * scale, masked by xy_mask
    C = A @ Kh^T, normalized per-row by 1/max(sum(mask, y), 1)
    out = wo @ (0.5 * (Q + C))

Sharding: data-parallel over batch; 16 batches / 8 cores = 2 per core.
Weights replicated. No cross-core communication.

Device algorithm (per core, fp16 compute with fp32 PSUM accumulation):
  - Q = wqT.T @ x, K = wkT.T @ y, KT = y.T @ wkT (natural-layout matmuls; the
    K transpose needed by the C-contraction is computed as a second projection
    instead of an on-chip transpose).
  - A is computed transposed (y on partitions) per head; relu+mask are fused
    into one DVE scalar_tensor_tensor: (A max 0) * maskT, or routed through
    ScalarE relu + DVE multiply when that balances engine load better.
    Two heads are packed in the 128-wide PE array (K=64 row groups for the
    A matmuls / M=64 col groups for the C matmuls); the two x-halves of each
    output share one weight load.
  - n_el row counts come from a ones^T @ maskT matmul; inv = 1/(8*max(n,1))
    folds the 1/sqrt(d) attention scale; 0.5 is folded into woT on the host.
"""
import sys

if "/opt/trn_rl_repo" not in sys.path:
    sys.path.insert(0, "/opt/trn_rl_repo")

import numpy as np

import concourse.tile as tile
from concourse import bacc, mybir
from concourse.bass_utils import run_bass_kernel_spmd

F16 = mybir.dt.float16
F32 = mybir.dt.float32
AL = mybir.AluOpType
RELU = mybir.ActivationFunctionType.Relu

N_CORES = 8
B, U, LX, LY, H, D = 16, 512, 1024, 1024, 8, 64
BPC = B // N_CORES          # batches per core
KB = U // 128               # 4  k-tiles over channels
HP = H // 2                 # 4  head pairs
YT = LY // 128              # 8  y tiles
XH = LX // 512              # 2  x halves
INV_SCALE = float(D) ** 0.5  # 8.0; attention scale = 1/8

TRACE = False
LDW_OPT = False
_CACHE = {}


def _patch_ldw_opt():
    """Flip walrus --enable-ldw-opt so back-to-back matmuls sharing a
    stationary operand don't reload the PE array each time."""
    from concourse import bass_utils as bu
    if getattr(bu.bir_verify_and_optimise, "_ldw_patched", False):
        return
    orig = bu.bir_verify_and_optimise

    def patched(*args, **kwargs):
        import subprocess
        orig_run = subprocess.run

        def run_hook(cmd, *a, **kw):
            if isinstance(cmd, list):
                cmd = ["--enable-ldw-opt=true" if c == "--enable-ldw-opt=false"
                       else c for c in cmd]
            return orig_run(cmd, *a, **kw)

        subprocess.run = run_hook
        try:
            return orig(*args, **kwargs)
        finally:
            subprocess.run = orig_run

    patched._ldw_patched = True
    bu.bir_verify_and_optimise = patched
    # bass2jax's hook imports the symbol through the module, patch is enough


class Balance:
    """Greedy static load balancer between DVE (vector) and ACT (scalar)."""

    def __init__(self, nc):
        self.nc = nc
        self.t = {"v": 0.0, "s": 0.0}

    DVE_FUDGE = 1.06

    def add(self, eng, ns):
        if eng == "v":
            ns *= self.DVE_FUDGE
        self.t[eng] += ns

    def copy(self, dst, src, fd):
        dve = (fd + 130) / 0.96 * self.DVE_FUDGE
        act = (fd + 340) / 1.2
        if self.t["v"] + dve <= self.t["s"] + act:
            self.t["v"] += dve
            self.nc.vector.tensor_copy(dst, src)
        else:
            self.t["s"] += act
            self.nc.scalar.copy(dst, src)

    def relu_mask_pair(self, out, a_ps, mtf_b, mtf2d, tmp_pool, name):
        """out[(128,2,512) f16] = relu(a_ps) * mtf broadcast over head dim.

        option 1: one fused DVE scalar_tensor_tensor at FD=1024;
        option 2: ScalarE relu at FD=1024 + two DVE f16 2x multiplies.
        """
        stt = (1024 + 190) / 0.96 * self.DVE_FUDGE
        act = (1024 + 340) / 1.2
        tt2 = (512 + 120) / 0.96 * self.DVE_FUDGE
        if self.t["v"] + stt <= max(self.t["s"] + act, self.t["v"] + tt2):
            self.t["v"] += stt
            self.nc.vector.scalar_tensor_tensor(out[:], a_ps[:], 0.0, mtf_b,
                                                AL.max, AL.mult)
        else:
            self.t["s"] += act
            self.t["v"] += tt2
            at = tmp_pool.tile([128, 2, 512], F16, tag="at", name=name)
            self.nc.scalar.activation(at[:], a_ps[:], RELU)
            self.nc.vector.tensor_tensor(out[:], at[:], mtf_b, AL.mult)


def _build():
    if LDW_OPT:
        _patch_ldw_opt()
    nc = bacc.Bacc("TRN2", target_bir_lowering=False, debug=False,
                   num_devices=N_CORES)
    x_e = nc.dram_tensor("x", [BPC, U, LX], F16, kind="ExternalInput")
    y_e = nc.dram_tensor("y", [BPC, U, LY], F16, kind="ExternalInput")
    mt_e = nc.dram_tensor("mt", [BPC, LY, LX], F16, kind="ExternalInput")
    w_all_e = nc.dram_tensor("w_all", [3, U, U], F16, kind="ExternalInput")
    o_e = nc.dram_tensor("o", [BPC, U, LX], F32, kind="ExternalOutput")

    with tile.TileContext(nc) as tc:
        _emit(nc, tc, x_e, y_e, mt_e, w_all_e, o_e)
    nc.compile()
    return nc


def _emit(nc, tc, x_e, y_e, mt_e, w_all_e, o_e):
    import contextlib
    bal = Balance(nc)
    ctx = contextlib.ExitStack()
    with ctx:
        wp = ctx.enter_context(tc.tile_pool(name="wp", bufs=1))
        io = ctx.enter_context(tc.tile_pool(name="io", bufs=2))
        pr = ctx.enter_context(tc.tile_pool(name="pr", bufs=2))
        sm = ctx.enter_context(tc.tile_pool(name="sm", bufs=2))
        amp = ctx.enter_context(tc.tile_pool(name="amp", bufs=4))
        osp = ctx.enter_context(tc.tile_pool(name="osp", bufs=4))
        pa = ctx.enter_context(tc.tile_pool(name="pa", bufs=3, space="PSUM"))
        pc = ctx.enter_context(tc.tile_pool(name="pc", bufs=2, space="PSUM"))

        # weights, loaded once
        WQT = wp.tile([128, KB, U], F16, tag="wqt")
        WKT = wp.tile([128, KB, U], F16, tag="wkt")
        WOT = wp.tile([128, KB, U], F16, tag="wot")
        for wi, w_t in enumerate((WQT, WKT, WOT)):
            nc.scalar.dma_start(
                w_t[:], w_all_e.ap()[wi].rearrange("(k p) o -> p k o", p=128))
        ones = wp.tile([128, 1], F16, tag="ones")
        nc.vector.memset(ones[:], 1.0)

        for b in range(BPC):
            # ---- input loads (split so compute can start early) ----
            X = io.tile([128, KB, LX], F16, tag="x", name=f"x{b}")
            Y = io.tile([128, KB, LY], F16, tag="y", name=f"y{b}")
            engs = [nc.sync, nc.scalar, nc.gpsimd]
            for k in range(KB):
                engs[k % 3].dma_start(X[:, k, :], x_e.ap()[b, k * 128:(k + 1) * 128, :])
            for k in range(KB):
                engs[(k + 1) % 3].dma_start(Y[:, k, :], y_e.ap()[b, k * 128:(k + 1) * 128, :])
            MTF = io.tile([128, YT, LX], F16, tag="mtf", name=f"mtf{b}")
            for t in range(YT):
                engs[(t + 2) % 3].dma_start(
                    MTF[:, t, :], mt_e.ap()[b, t * 128:(t + 1) * 128, :])

            # ---- projections: Q = wqT.T @ x, K = wkT.T @ y ----
            Q = pr.tile([128, KB, LX], F16, tag="q", name=f"q{b}")
            K = pr.tile([128, KB, LY], F16, tag="k", name=f"k{b}")
            for w_t, src, dst in ((WQT, X, Q), (WKT, Y, K)):
                for m in range(KB):
                    ps = pa.tile([128, 2, 512], F32, tag="a",
                                 name=f"pj{b}_{dst.name}_{m}")
                    for k in range(KB):
                        for n in range(XH):
                            nc.tensor.matmul(
                                ps[:, n, :], w_t[:, k, m * 128:(m + 1) * 128],
                                src[:, k, n * 512:(n + 1) * 512],
                                start=(k == 0), stop=(k == KB - 1))
                    bal.copy(dst[:, m, :], ps[:], 1024)
            # KT = y.T @ wkT : two l-tiles share one psum pair slot
            KT = pr.tile([128, YT, U], F16, tag="kt", name=f"kt{b}")
            for lt2 in range(YT // 2):
                ps = pa.tile([128, 2, 512], F32, tag="a", name=f"pkt{b}_{lt2}")
                for i in range(2):
                    lt = lt2 * 2 + i
                    for k in range(KB):
                        nc.tensor.matmul(ps[:, i, :],
                                         Y[:, k, lt * 128:(lt + 1) * 128],
                                         WKT[:, k, :512],
                                         start=(k == 0), stop=(k == KB - 1))
                bal.copy(KT[:, lt2 * 2:lt2 * 2 + 2, :], ps[:], 1024)

            # ---- mask row counts and inverse ----
            nel = pa.tile([1, 2, 512], F32, tag="a", name=f"nel{b}")
            for yt in range(YT):
                for xh in range(XH):
                    nc.tensor.matmul(nel[0:1, xh, :], ones[:],
                                     MTF[:, yt, xh * 512:(xh + 1) * 512],
                                     start=(yt == 0), stop=(yt == YT - 1))
            nelc = sm.tile([1, LX], F32, tag="nelc", name=f"nelc{b}")
            nc.vector.tensor_scalar(nelc[:], nel[:], 1.0, INV_SCALE, AL.max, AL.mult)
            bal.add("v", 1200)
            # reciprocal is ~8 cycles/free-element on DVE; bounce through a
            # (128 x 8) layout so the iteration count is 8, not 1024
            nelp = sm.tile([128, 8], F32, tag="nelp", name=f"nelp{b}")
            nc.sync.dma_start(nelp[:], nelc[:])
            invp = sm.tile([128, 8], F32, tag="invp", name=f"invp{b}")
            nc.vector.reciprocal(invp[:], nelp[:])
            bal.add("v", 300)
            invr = sm.tile([1, LX], F32, tag="invr", name=f"invr{b}")
            nc.sync.dma_start(invr[:], invp[:])
            invb = sm.tile([128, LX], F32, tag="invb", name=f"invb{b}")
            nc.gpsimd.partition_broadcast(invb[:], invr[:])

            # ---- attention ----
            E = pr.tile([128, KB, LX], F16, tag="e", name=f"e{b}")
            for hp in range(HP):
                for xh in range(XH):
                    xs = slice(xh * 512, (xh + 1) * 512)
                    # both heads accumulate into ONE bank: j0 at partitions
                    # 0-63 (col group 0), j1 at 64-127 (col group 64). The
                    # has_written clear on group start is per-M-range, so the
                    # two groups coexist; same-bank drain merges the pair's
                    # column writes into one PSUM port transaction.
                    C = pc.tile([128, 512], F32, tag="c", name=f"c_{b}_{hp}_{xh}")
                    for yt in range(YT):
                        A = pa.tile([128, 2, 512], F32, tag="a",
                                    name=f"a_{b}_{hp}_{xh}_{yt}")
                        for j in range(2):
                            hs = slice(64 * j, 64 * (j + 1))
                            nc.tensor.matmul(
                                A[:, j, :], K[hs, hp, yt * 128:(yt + 1) * 128],
                                Q[hs, hp, xs], start=True, stop=True)
                        Am = amp.tile([128, 2, 512], F16, tag="am",
                                      name=f"am_{b}_{hp}_{xh}_{yt}")
                        mtf_b = MTF[:, yt, xs].unsqueeze(1).broadcast_to((128, 2, 512))
                        bal.relu_mask_pair(Am, A, mtf_b, MTF[:, yt, xs], amp,
                                           f"at_{b}_{hp}_{xh}_{yt}")
                        for j in range(2):
                            hs = slice(64 * j, 64 * (j + 1))
                            nc.tensor.matmul(
                                C[hs, :],
                                KT[:, yt, hp * 128 + 64 * j: hp * 128 + 64 * (j + 1)],
                                Am[:, j, :], start=(yt == 0), stop=(yt == YT - 1),
                                skip_group_check=True)
                    Et = amp.tile([128, 512], F16, tag="et", name=f"et_{b}_{hp}_{xh}")
                    nc.vector.tensor_tensor(Et[:], C[:], invb[:, xs], AL.mult)
                    bal.add("v", 670)
                    nc.vector.tensor_tensor(E[:, hp, xs], Et[:], Q[:, hp, xs], AL.add)
                    bal.add("v", 330)

            # ---- output projection ----
            for m in range(KB):
                ps = pa.tile([128, 2, 512], F32, tag="a", name=f"po{b}_{m}")
                for k in range(KB):
                    for n in range(XH):
                        nc.tensor.matmul(ps[:, n, :],
                                         WOT[:, k, m * 128:(m + 1) * 128],
                                         E[:, k, n * 512:(n + 1) * 512],
                                         start=(k == 0), stop=(k == KB - 1))
                oS = osp.tile([128, LX], F32, tag="os", name=f"os{b}_{m}")
                for n in range(XH):
                    bal.copy(oS[:, n * 512:(n + 1) * 512], ps[:, n, :], 512)
                    nc.sync.dma_start(
                        o_e.ap()[b, m * 128:(m + 1) * 128, n * 512:(n + 1) * 512],
                        oS[:, n * 512:(n + 1) * 512])


def _get_nc():
    if "nc" not in _CACHE:
        _CACHE["nc"] = _build()
    return _CACHE["nc"]


def kernel(x, y, xy_mask, wq, wk, wo):
    nc = _get_nc()
    xf = x.astype(np.float16)
    yf = y.astype(np.float16)
    mtt = np.ascontiguousarray(
        xy_mask.transpose(0, 2, 1)).astype(np.float16)
    w_all = np.stack([wq.T, wk.T, (0.5 * wo).T]).astype(np.float16)
    w_all = np.ascontiguousarray(w_all)
    in_maps = [
        {"x": xf[c * BPC:(c + 1) * BPC], "y": yf[c * BPC:(c + 1) * BPC],
         "mt": mtt[c * BPC:(c + 1) * BPC], "w_all": w_all}
        for c in range(N_CORES)
    ]
    res = run_bass_kernel_spmd(nc, in_maps, list(range(N_CORES)), trace=TRACE)
    if TRACE:
        _CACHE["last_exec_time_ns"] = res.exec_time_ns
        _CACHE["last_profile_json"] = res.profile_json
    return np.concatenate([res.results[c]["o"] for c in range(N_CORES)], axis=0)


# revision 17
# speedup vs baseline: 1.0440x; 1.0440x over previous
"""MultiHeadSimilarity kernel for 8 Trainium2 NeuronCores.

Reference computation (per batch b):
    Q = wq @ x[b];  K = wk @ y[b]                       (channel-mixing matmuls)
    per head h (d=64):  A = relu(Qh^T Kh) * scale, masked by xy_mask
    C = A @ Kh^T, normalized per-row by 1/max(sum(mask, y), 1)
    out = wo @ (0.5 * (Q + C))

Sharding: data-parallel over batch; 16 batches / 8 cores = 2 per core.
Weights replicated. No cross-core communication.

Device algorithm (per core, fp16 compute with fp32 PSUM accumulation):
  - Q = wqT.T @ x, K = wkT.T @ y, KT = y.T @ wkT (natural-layout matmuls; the
    K transpose needed by the C-contraction is computed as a second projection
    instead of an on-chip transpose).
  - A is computed transposed (y on partitions) per head; relu+mask are fused
    into one DVE scalar_tensor_tensor: (A max 0) * maskT, or routed through
    ScalarE relu + DVE multiply when that balances engine load better.
    Two heads are packed in the 128-wide PE array (K=64 row groups for the
    A matmuls / M=64 col groups for the C matmuls); the two x-halves of each
    output share one weight load.
  - n_el row counts come from a ones^T @ maskT matmul; inv = 1/(8*max(n,1))
    folds the 1/sqrt(d) attention scale; 0.5 is folded into woT on the host.
"""
import sys

if "/opt/trn_rl_repo" not in sys.path:
    sys.path.insert(0, "/opt/trn_rl_repo")

import numpy as np

import concourse.tile as tile
from concourse import bacc, mybir
from concourse.bass_utils import run_bass_kernel_spmd

F16 = mybir.dt.float16
F32 = mybir.dt.float32
AL = mybir.AluOpType
RELU = mybir.ActivationFunctionType.Relu

N_CORES = 8
B, U, LX, LY, H, D = 16, 512, 1024, 1024, 8, 64
BPC = B // N_CORES          # batches per core
KB = U // 128               # 4  k-tiles over channels
HP = H // 2                 # 4  head pairs
YT = LY // 128              # 8  y tiles
XH = LX // 512              # 2  x halves
INV_SCALE = float(D) ** 0.5  # 8.0; attention scale = 1/8

TRACE = False
LDW_OPT = False
_CACHE = {}


def _patch_ldw_opt():
    """Flip walrus --enable-ldw-opt so back-to-back matmuls sharing a
    stationary operand don't reload the PE array each time."""
    from concourse import bass_utils as bu
    if getattr(bu.bir_verify_and_optimise, "_ldw_patched", False):
        return
    orig = bu.bir_verify_and_optimise

    def patched(*args, **kwargs):
        import subprocess
        orig_run = subprocess.run

        def run_hook(cmd, *a, **kw):
            if isinstance(cmd, list):
                cmd = ["--enable-ldw-opt=true" if c == "--enable-ldw-opt=false"
                       else c for c in cmd]
            return orig_run(cmd, *a, **kw)

        subprocess.run = run_hook
        try:
            return orig(*args, **kwargs)
        finally:
            subprocess.run = orig_run

    patched._ldw_patched = True
    bu.bir_verify_and_optimise = patched
    # bass2jax's hook imports the symbol through the module, patch is enough


class Balance:
    """Greedy static load balancer between DVE (vector) and ACT (scalar)."""

    def __init__(self, nc):
        self.nc = nc
        self.t = {"v": 0.0, "s": 0.0}

    DVE_FUDGE = 1.06

    def add(self, eng, ns):
        if eng == "v":
            ns *= self.DVE_FUDGE
        self.t[eng] += ns

    def copy(self, dst, src, fd):
        dve = (fd + 130) / 0.96 * self.DVE_FUDGE
        act = (fd + 340) / 1.2
        if self.t["v"] + dve <= self.t["s"] + act:
            self.t["v"] += dve
            self.nc.vector.tensor_copy(dst, src)
        else:
            self.t["s"] += act
            self.nc.scalar.copy(dst, src)

    def relu_mask_pair(self, out, a_ps, mtf_b, mtf2d, tmp_pool, name):
        """out[(128,2,512) f16] = relu(a_ps) * mtf broadcast over head dim.

        option 1: one fused DVE scalar_tensor_tensor at FD=1024;
        option 2: ScalarE relu at FD=1024 + two DVE f16 2x multiplies.
        """
        stt = (1024 + 190) / 0.96 * self.DVE_FUDGE
        act = (1024 + 340) / 1.2
        tt2 = (512 + 120) / 0.96 * self.DVE_FUDGE
        if self.t["v"] + stt <= max(self.t["s"] + act, self.t["v"] + tt2):
            self.t["v"] += stt
            self.nc.vector.scalar_tensor_tensor(out[:], a_ps[:], 0.0, mtf_b,
                                                AL.max, AL.mult)
        else:
            self.t["s"] += act
            self.t["v"] += tt2
            at = tmp_pool.tile([128, 2, 512], F16, tag="at", name=name)
            self.nc.scalar.activation(at[:], a_ps[:], RELU)
            self.nc.vector.tensor_tensor(out[:], at[:], mtf_b, AL.mult)


def _build():
    if LDW_OPT:
        _patch_ldw_opt()
    nc = bacc.Bacc("TRN2", target_bir_lowering=False, debug=False,
                   num_devices=N_CORES)
    x_e = nc.dram_tensor("x", [BPC, U, LX], F16, kind="ExternalInput")
    y_e = nc.dram_tensor("y", [BPC, U, LY], F16, kind="ExternalInput")
    mt_e = nc.dram_tensor("mt", [BPC, LY, LX], F16, kind="ExternalInput")
    w_all_e = nc.dram_tensor("w_all", [3, U, U], F16, kind="ExternalInput")
    o_e = nc.dram_tensor("o", [BPC, U, LX], F32, kind="ExternalOutput")

    with tile.TileContext(nc) as tc:
        _emit(nc, tc, x_e, y_e, mt_e, w_all_e, o_e)
    nc.compile()
    return nc


def _emit(nc, tc, x_e, y_e, mt_e, w_all_e, o_e):
    import contextlib
    bal = Balance(nc)
    ctx = contextlib.ExitStack()
    with ctx:
        wp = ctx.enter_context(tc.tile_pool(name="wp", bufs=1))
        io = ctx.enter_context(tc.tile_pool(name="io", bufs=2))
        pr = ctx.enter_context(tc.tile_pool(name="pr", bufs=2))
        sm = ctx.enter_context(tc.tile_pool(name="sm", bufs=2))
        amp = ctx.enter_context(tc.tile_pool(name="amp", bufs=4))
        osp = ctx.enter_context(tc.tile_pool(name="osp", bufs=4))
        pa = ctx.enter_context(tc.tile_pool(name="pa", bufs=3, space="PSUM"))
        pc = ctx.enter_context(tc.tile_pool(name="pc", bufs=2, space="PSUM"))

        # weights, loaded once
        WQT = wp.tile([128, KB, U], F16, tag="wqt")
        WKT = wp.tile([128, KB, U], F16, tag="wkt")
        WOT = wp.tile([128, KB, U], F16, tag="wot")
        for wi, w_t in enumerate((WQT, WKT, WOT)):
            nc.scalar.dma_start(
                w_t[:], w_all_e.ap()[wi].rearrange("(k p) o -> p k o", p=128))
        ones = wp.tile([128, 1], F16, tag="ones")
        nc.vector.memset(ones[:], 1.0)

        for b in range(BPC):
            # ---- input loads (split so compute can start early) ----
            X = io.tile([128, KB, LX], F16, tag="x", name=f"x{b}")
            Y = io.tile([128, KB, LY], F16, tag="y", name=f"y{b}")
            for k in range(KB):
                nc.sync.dma_start(X[:, k, :], x_e.ap()[b, k * 128:(k + 1) * 128, :])
            for k in range(KB):
                nc.gpsimd.dma_start(Y[:, k, :], y_e.ap()[b, k * 128:(k + 1) * 128, :])
            MTF = io.tile([128, YT, LX], F16, tag="mtf", name=f"mtf{b}")
            for t in range(YT):
                (nc.sync if t % 2 == 0 else nc.gpsimd).dma_start(
                    MTF[:, t, :], mt_e.ap()[b, t * 128:(t + 1) * 128, :])

            # ---- projections: Q = wqT.T @ x, K = wkT.T @ y ----
            Q = pr.tile([128, KB, LX], F16, tag="q", name=f"q{b}")
            K = pr.tile([128, KB, LY], F16, tag="k", name=f"k{b}")
            for w_t, src, dst in ((WQT, X, Q), (WKT, Y, K)):
                for m in range(KB):
                    ps = pa.tile([128, 2, 512], F32, tag="a",
                                 name=f"pj{b}_{dst.name}_{m}")
                    for k in range(KB):
                        for n in range(XH):
                            nc.tensor.matmul(
                                ps[:, n, :], w_t[:, k, m * 128:(m + 1) * 128],
                                src[:, k, n * 512:(n + 1) * 512],
                                start=(k == 0), stop=(k == KB - 1))
                    bal.copy(dst[:, m, :], ps[:], 1024)
            # KT = y.T @ wkT : two l-tiles share one psum pair slot
            KT = pr.tile([128, YT, U], F16, tag="kt", name=f"kt{b}")
            for lt2 in range(YT // 2):
                ps = pa.tile([128, 2, 512], F32, tag="a", name=f"pkt{b}_{lt2}")
                for i in range(2):
                    lt = lt2 * 2 + i
                    for k in range(KB):
                        nc.tensor.matmul(ps[:, i, :],
                                         Y[:, k, lt * 128:(lt + 1) * 128],
                                         WKT[:, k, :512],
                                         start=(k == 0), stop=(k == KB - 1))
                bal.copy(KT[:, lt2 * 2:lt2 * 2 + 2, :], ps[:], 1024)

            # ---- mask row counts and inverse ----
            nel = pa.tile([1, 2, 512], F32, tag="a", name=f"nel{b}")
            for yt in range(YT):
                for xh in range(XH):
                    nc.tensor.matmul(nel[0:1, xh, :], ones[:],
                                     MTF[:, yt, xh * 512:(xh + 1) * 512],
                                     start=(yt == 0), stop=(yt == YT - 1))
            nelc = sm.tile([1, LX], F32, tag="nelc", name=f"nelc{b}")
            nc.vector.tensor_scalar(nelc[:], nel[:], 1.0, INV_SCALE, AL.max, AL.mult)
            bal.add("v", 1200)
            # reciprocal is ~8 cycles/free-element on DVE; bounce through a
            # (128 x 8) layout so the iteration count is 8, not 1024
            nelp = sm.tile([128, 8], F32, tag="nelp", name=f"nelp{b}")
            nc.sync.dma_start(nelp[:], nelc[:])
            invp = sm.tile([128, 8], F32, tag="invp", name=f"invp{b}")
            nc.vector.reciprocal(invp[:], nelp[:])
            bal.add("v", 300)
            invr = sm.tile([1, LX], F32, tag="invr", name=f"invr{b}")
            nc.sync.dma_start(invr[:], invp[:])
            invb = sm.tile([128, LX], F32, tag="invb", name=f"invb{b}")
            nc.gpsimd.partition_broadcast(invb[:], invr[:])

            # ---- attention ----
            E = pr.tile([128, KB, LX], F16, tag="e", name=f"e{b}")
            for hp in range(HP):
                for xh in range(XH):
                    xs = slice(xh * 512, (xh + 1) * 512)
                    # both heads accumulate into ONE bank: j0 at partitions
                    # 0-63 (col group 0), j1 at 64-127 (col group 64). The
                    # has_written clear on group start is per-M-range, so the
                    # two groups coexist; same-bank drain merges the pair's
                    # column writes into one PSUM port transaction.
                    C = pc.tile([128, 512], F32, tag="c", name=f"c_{b}_{hp}_{xh}")
                    for yt in range(YT):
                        A = pa.tile([128, 2, 512], F32, tag="a",
                                    name=f"a_{b}_{hp}_{xh}_{yt}")
                        for j in range(2):
                            hs = slice(64 * j, 64 * (j + 1))
                            nc.tensor.matmul(
                                A[:, j, :], K[hs, hp, yt * 128:(yt + 1) * 128],
                                Q[hs, hp, xs], start=True, stop=True)
                        Am = amp.tile([128, 2, 512], F16, tag="am",
                                      name=f"am_{b}_{hp}_{xh}_{yt}")
                        mtf_b = MTF[:, yt, xs].unsqueeze(1).broadcast_to((128, 2, 512))
                        bal.relu_mask_pair(Am, A, mtf_b, MTF[:, yt, xs], amp,
                                           f"at_{b}_{hp}_{xh}_{yt}")
                        for j in range(2):
                            hs = slice(64 * j, 64 * (j + 1))
                            nc.tensor.matmul(
                                C[hs, :],
                                KT[:, yt, hp * 128 + 64 * j: hp * 128 + 64 * (j + 1)],
                                Am[:, j, :], start=(yt == 0), stop=(yt == YT - 1),
                                skip_group_check=True)
                    Et = amp.tile([128, 512], F16, tag="et", name=f"et_{b}_{hp}_{xh}")
                    nc.vector.tensor_tensor(Et[:], C[:], invb[:, xs], AL.mult)
                    bal.add("v", 670)
                    nc.vector.tensor_tensor(E[:, hp, xs], Et[:], Q[:, hp, xs], AL.add)
                    bal.add("v", 330)

            # ---- output projection ----
            for m in range(KB):
                ps = pa.tile([128, 2, 512], F32, tag="a", name=f"po{b}_{m}")
                for k in range(KB):
                    for n in range(XH):
                        nc.tensor.matmul(ps[:, n, :],
                                         WOT[:, k, m * 128:(m + 1) * 128],
                                         E[:, k, n * 512:(n + 1) * 512],
                                         start=(k == 0), stop=(k == KB - 1))
                oS = osp.tile([128, LX], F32, tag="os", name=f"os{b}_{m}")
                for n in range(XH):
                    bal.copy(oS[:, n * 512:(n + 1) * 512], ps[:, n, :], 512)
                    nc.sync.dma_start(
                        o_e.ap()[b, m * 128:(m + 1) * 128, n * 512:(n + 1) * 512],
                        oS[:, n * 512:(n + 1) * 512])


def _get_nc():
    if "nc" not in _CACHE:
        _CACHE["nc"] = _build()
    return _CACHE["nc"]


def kernel(x, y, xy_mask, wq, wk, wo):
    nc = _get_nc()
    xf = x.astype(np.float16)
    yf = y.astype(np.float16)
    mtt = np.ascontiguousarray(
        xy_mask.transpose(0, 2, 1)).astype(np.float16)
    w_all = np.stack([wq.T, wk.T, (0.5 * wo).T]).astype(np.float16)
    w_all = np.ascontiguousarray(w_all)
    in_maps = [
        {"x": xf[c * BPC:(c + 1) * BPC], "y": yf[c * BPC:(c + 1) * BPC],
         "mt": mtt[c * BPC:(c + 1) * BPC], "w_all": w_all}
        for c in range(N_CORES)
    ]
    res = run_bass_kernel_spmd(nc, in_maps, list(range(N_CORES)), trace=TRACE)
    if TRACE:
        _CACHE["last_exec_time_ns"] = res.exec_time_ns
        _CACHE["last_profile_json"] = res.profile_json
    return np.concatenate([res.results[c]["o"] for c in range(N_CORES)], axis=0)


# revision 19
# speedup vs baseline: 1.0511x; 1.0068x over previous
"""MultiHeadSimilarity kernel for 8 Trainium2 NeuronCores.

Reference computation (per batch b):
    Q = wq @ x[b];  K = wk @ y[b]                       (channel-mixing matmuls)
    per head h (d=64):  A = relu(Qh^T Kh) * scale, masked by xy_mask
    C = A @ Kh^T, normalized per-row by 1/max(sum(mask, y), 1)
    out = wo @ (0.5 * (Q + C))

Sharding: data-parallel over batch; 16 batches / 8 cores = 2 per core.
Weights replicated. No cross-core communication.

Device algorithm (per core, fp16 compute with fp32 PSUM accumulation):
  - Q = wqT.T @ x, K = wkT.T @ y, KT = y.T @ wkT (natural-layout matmuls; the
    K transpose needed by the C-contraction is computed as a second projection
    instead of an on-chip transpose).
  - A is computed transposed (y on partitions) per head; relu+mask are fused
    into one DVE scalar_tensor_tensor: (A max 0) * maskT, or routed through
    ScalarE relu + DVE multiply when that balances engine load better.
    Two heads are packed in the 128-wide PE array (K=64 row groups for the
    A matmuls / M=64 col groups for the C matmuls); the two x-halves of each
    output share one weight load.
  - n_el row counts come from a ones^T @ maskT matmul; inv = 1/(8*max(n,1))
    folds the 1/sqrt(d) attention scale; 0.5 is folded into woT on the host.
"""
import sys

if "/opt/trn_rl_repo" not in sys.path:
    sys.path.insert(0, "/opt/trn_rl_repo")

import numpy as np

import concourse.tile as tile
from concourse import bacc, mybir
from concourse.bass_utils import run_bass_kernel_spmd

F16 = mybir.dt.float16
F32 = mybir.dt.float32
AL = mybir.AluOpType
RELU = mybir.ActivationFunctionType.Relu

N_CORES = 8
B, U, LX, LY, H, D = 16, 512, 1024, 1024, 8, 64
BPC = B // N_CORES          # batches per core
KB = U // 128               # 4  k-tiles over channels
HP = H // 2                 # 4  head pairs
YT = LY // 128              # 8  y tiles
XH = LX // 512              # 2  x halves
INV_SCALE = float(D) ** 0.5  # 8.0; attention scale = 1/8

TRACE = False
LDW_OPT = False
_CACHE = {}


def _patch_ldw_opt():
    """Flip walrus --enable-ldw-opt so back-to-back matmuls sharing a
    stationary operand don't reload the PE array each time."""
    from concourse import bass_utils as bu
    if getattr(bu.bir_verify_and_optimise, "_ldw_patched", False):
        return
    orig = bu.bir_verify_and_optimise

    def patched(*args, **kwargs):
        import subprocess
        orig_run = subprocess.run

        def run_hook(cmd, *a, **kw):
            if isinstance(cmd, list):
                cmd = ["--enable-ldw-opt=true" if c == "--enable-ldw-opt=false"
                       else c for c in cmd]
            return orig_run(cmd, *a, **kw)

        subprocess.run = run_hook
        try:
            return orig(*args, **kwargs)
        finally:
            subprocess.run = orig_run

    patched._ldw_patched = True
    bu.bir_verify_and_optimise = patched
    # bass2jax's hook imports the symbol through the module, patch is enough


class Balance:
    """Greedy static load balancer between DVE (vector) and ACT (scalar)."""

    def __init__(self, nc):
        self.nc = nc
        self.t = {"v": 0.0, "s": 0.0}

    DVE_FUDGE = 1.06

    def add(self, eng, ns):
        if eng == "v":
            ns *= self.DVE_FUDGE
        self.t[eng] += ns

    def copy(self, dst, src, fd):
        dve = (fd + 130) / 0.96 * self.DVE_FUDGE
        act = (fd + 340) / 1.2
        if self.t["v"] + dve <= self.t["s"] + act:
            self.t["v"] += dve
            self.nc.vector.tensor_copy(dst, src)
        else:
            self.t["s"] += act
            self.nc.scalar.copy(dst, src)

    def relu_mask_pair(self, out, a_ps, mtf_b, mtf2d, tmp_pool, name):
        """out[(128,2,512) f16] = relu(a_ps) * mtf broadcast over head dim.

        option 1: one fused DVE scalar_tensor_tensor at FD=1024;
        option 2: ScalarE relu at FD=1024 + two DVE f16 2x multiplies.
        """
        stt = (1024 + 190) / 0.96 * self.DVE_FUDGE
        act = (1024 + 340) / 1.2
        tt2 = (512 + 120) / 0.96 * self.DVE_FUDGE
        if self.t["v"] + stt <= max(self.t["s"] + act, self.t["v"] + tt2):
            self.t["v"] += stt
            self.nc.vector.scalar_tensor_tensor(out[:], a_ps[:], 0.0, mtf_b,
                                                AL.max, AL.mult)
        else:
            self.t["s"] += act
            self.t["v"] += tt2
            at = tmp_pool.tile([128, 2, 512], F16, tag="at", name=name)
            self.nc.scalar.activation(at[:], a_ps[:], RELU)
            self.nc.vector.tensor_tensor(out[:], at[:], mtf_b, AL.mult)


def _build():
    if LDW_OPT:
        _patch_ldw_opt()
    nc = bacc.Bacc("TRN2", target_bir_lowering=False, debug=False,
                   num_devices=N_CORES)
    x_e = nc.dram_tensor("x", [BPC, U, LX], F16, kind="ExternalInput")
    y_e = nc.dram_tensor("y", [BPC, U, LY], F16, kind="ExternalInput")
    mt_e = nc.dram_tensor("mt", [BPC, LY, LX], F16, kind="ExternalInput")
    w_all_e = nc.dram_tensor("w_all", [3, U, U], F16, kind="ExternalInput")
    o_e = nc.dram_tensor("o", [BPC, U, LX], F32, kind="ExternalOutput")

    with tile.TileContext(nc) as tc:
        _emit(nc, tc, x_e, y_e, mt_e, w_all_e, o_e)
    nc.compile()
    return nc


def _emit(nc, tc, x_e, y_e, mt_e, w_all_e, o_e):
    import contextlib
    bal = Balance(nc)
    ctx = contextlib.ExitStack()
    with ctx:
        wp = ctx.enter_context(tc.tile_pool(name="wp", bufs=1))
        io = ctx.enter_context(tc.tile_pool(name="io", bufs=2))
        pr = ctx.enter_context(tc.tile_pool(name="pr", bufs=2))
        sm = ctx.enter_context(tc.tile_pool(name="sm", bufs=2))
        amp = ctx.enter_context(tc.tile_pool(name="amp", bufs=4))
        osp = ctx.enter_context(tc.tile_pool(name="osp", bufs=3))
        pa = ctx.enter_context(tc.tile_pool(name="pa", bufs=3, space="PSUM"))
        pc = ctx.enter_context(tc.tile_pool(name="pc", bufs=2, space="PSUM"))

        # weights, loaded once
        WQT = wp.tile([128, KB, U], F16, tag="wqt")
        WKT = wp.tile([128, KB, U], F16, tag="wkt")
        WOT = wp.tile([128, KB, U], F16, tag="wot")
        for wi, w_t in enumerate((WQT, WKT, WOT)):
            nc.scalar.dma_start(
                w_t[:], w_all_e.ap()[wi].rearrange("(k p) o -> p k o", p=128))
        ones = wp.tile([128, 1], F16, tag="ones")
        nc.vector.memset(ones[:], 1.0)

        for b in range(BPC):
            # ---- input loads (split so compute can start early) ----
            X = io.tile([128, KB, LX], F16, tag="x", name=f"x{b}")
            Y = io.tile([128, KB, LY], F16, tag="y", name=f"y{b}")
            for k in range(KB):
                nc.sync.dma_start(X[:, k, :], x_e.ap()[b, k * 128:(k + 1) * 128, :])
            for k in range(KB):
                nc.gpsimd.dma_start(Y[:, k, :], y_e.ap()[b, k * 128:(k + 1) * 128, :])
            MTF = io.tile([128, YT, LX], F16, tag="mtf", name=f"mtf{b}")
            for t in range(YT):
                (nc.sync if t % 2 == 0 else nc.gpsimd).dma_start(
                    MTF[:, t, :], mt_e.ap()[b, t * 128:(t + 1) * 128, :])

            # ---- projections: Q = wqT.T @ x, K = wkT.T @ y ----
            Q = pr.tile([128, KB, LX], F16, tag="q", name=f"q{b}")
            K = pr.tile([128, KB, LY], F16, tag="k", name=f"k{b}")
            for w_t, src, dst in ((WQT, X, Q), (WKT, Y, K)):
                for m in range(KB):
                    ps = pa.tile([128, 2, 512], F32, tag="a",
                                 name=f"pj{b}_{dst.name}_{m}")
                    for k in range(KB):
                        for n in range(XH):
                            nc.tensor.matmul(
                                ps[:, n, :], w_t[:, k, m * 128:(m + 1) * 128],
                                src[:, k, n * 512:(n + 1) * 512],
                                start=(k == 0), stop=(k == KB - 1))
                    bal.copy(dst[:, m, :], ps[:], 1024)
            # KT = y.T @ wkT : two l-tiles share one psum pair slot
            KT = pr.tile([128, YT, U], F16, tag="kt", name=f"kt{b}")
            for lt2 in range(YT // 2):
                ps = pa.tile([128, 2, 512], F32, tag="a", name=f"pkt{b}_{lt2}")
                for i in range(2):
                    lt = lt2 * 2 + i
                    for k in range(KB):
                        nc.tensor.matmul(ps[:, i, :],
                                         Y[:, k, lt * 128:(lt + 1) * 128],
                                         WKT[:, k, :512],
                                         start=(k == 0), stop=(k == KB - 1))
                bal.copy(KT[:, lt2 * 2:lt2 * 2 + 2, :], ps[:], 1024)

            # ---- mask row counts and inverse ----
            nel = pa.tile([1, 2, 512], F32, tag="a", name=f"nel{b}")
            for yt in range(YT):
                for xh in range(XH):
                    nc.tensor.matmul(nel[0:1, xh, :], ones[:],
                                     MTF[:, yt, xh * 512:(xh + 1) * 512],
                                     start=(yt == 0), stop=(yt == YT - 1))
            nelc = sm.tile([1, LX], F32, tag="nelc", name=f"nelc{b}")
            nc.vector.tensor_scalar(nelc[:], nel[:], 1.0, INV_SCALE, AL.max, AL.mult)
            bal.add("v", 1200)
            # reciprocal is ~8 cycles/free-element on DVE; bounce through a
            # (128 x 8) layout so the iteration count is 8, not 1024
            nelp = sm.tile([128, 8], F32, tag="nelp", name=f"nelp{b}")
            nc.sync.dma_start(nelp[:], nelc[:])
            invp = sm.tile([128, 8], F32, tag="invp", name=f"invp{b}")
            nc.vector.reciprocal(invp[:], nelp[:])
            bal.add("v", 300)
            invr = sm.tile([1, LX], F32, tag="invr", name=f"invr{b}")
            nc.sync.dma_start(invr[:], invp[:])
            invb = sm.tile([128, LX], F32, tag="invb", name=f"invb{b}")
            nc.gpsimd.partition_broadcast(invb[:], invr[:])

            # ---- attention ----
            E = pr.tile([128, KB, LX], F16, tag="e", name=f"e{b}")
            for hp in range(HP):
                for xh in range(XH):
                    xs = slice(xh * 512, (xh + 1) * 512)
                    # both heads accumulate into ONE bank: j0 at partitions
                    # 0-63 (col group 0), j1 at 64-127 (col group 64). The
                    # has_written clear on group start is per-M-range, so the
                    # two groups coexist; same-bank drain merges the pair's
                    # column writes into one PSUM port transaction.
                    C = pc.tile([128, 512], F32, tag="c", name=f"c_{b}_{hp}_{xh}")
                    for yt in range(YT):
                        A = pa.tile([128, 2, 512], F32, tag="a",
                                    name=f"a_{b}_{hp}_{xh}_{yt}")
                        for j in range(2):
                            hs = slice(64 * j, 64 * (j + 1))
                            nc.tensor.matmul(
                                A[:, j, :], K[hs, hp, yt * 128:(yt + 1) * 128],
                                Q[hs, hp, xs], start=True, stop=True)
                        Am = amp.tile([128, 2, 512], F16, tag="am", bufs=6,
                                      name=f"am_{b}_{hp}_{xh}_{yt}")
                        mtf_b = MTF[:, yt, xs].unsqueeze(1).broadcast_to((128, 2, 512))
                        bal.relu_mask_pair(Am, A, mtf_b, MTF[:, yt, xs], amp,
                                           f"at_{b}_{hp}_{xh}_{yt}")
                        for j in range(2):
                            hs = slice(64 * j, 64 * (j + 1))
                            nc.tensor.matmul(
                                C[hs, :],
                                KT[:, yt, hp * 128 + 64 * j: hp * 128 + 64 * (j + 1)],
                                Am[:, j, :], start=(yt == 0), stop=(yt == YT - 1),
                                skip_group_check=True)
                    Et = amp.tile([128, 512], F16, tag="et", name=f"et_{b}_{hp}_{xh}")
                    nc.vector.tensor_tensor(Et[:], C[:], invb[:, xs], AL.mult)
                    bal.add("v", 670)
                    nc.vector.tensor_tensor(E[:, hp, xs], Et[:], Q[:, hp, xs], AL.add)
                    bal.add("v", 330)

            # ---- output projection ----
            for m in range(KB):
                ps = pa.tile([128, 2, 512], F32, tag="a", name=f"po{b}_{m}")
                for k in range(KB):
                    for n in range(XH):
                        nc.tensor.matmul(ps[:, n, :],
                                         WOT[:, k, m * 128:(m + 1) * 128],
                                         E[:, k, n * 512:(n + 1) * 512],
                                         start=(k == 0), stop=(k == KB - 1))
                oS = osp.tile([128, LX], F32, tag="os", name=f"os{b}_{m}")
                for n in range(XH):
                    bal.copy(oS[:, n * 512:(n + 1) * 512], ps[:, n, :], 512)
                    nc.sync.dma_start(
                        o_e.ap()[b, m * 128:(m + 1) * 128, n * 512:(n + 1) * 512],
                        oS[:, n * 512:(n + 1) * 512])


def _get_nc():
    if "nc" not in _CACHE:
        _CACHE["nc"] = _build()
    return _CACHE["nc"]


def kernel(x, y, xy_mask, wq, wk, wo):
    nc = _get_nc()
    xf = x.astype(np.float16)
    yf = y.astype(np.float16)
    mtt = np.ascontiguousarray(
        xy_mask.transpose(0, 2, 1)).astype(np.float16)
    w_all = np.stack([wq.T, wk.T, (0.5 * wo).T]).astype(np.float16)
    w_all = np.ascontiguousarray(w_all)
    in_maps = [
        {"x": xf[c * BPC:(c + 1) * BPC], "y": yf[c * BPC:(c + 1) * BPC],
         "mt": mtt[c * BPC:(c + 1) * BPC], "w_all": w_all}
        for c in range(N_CORES)
    ]
    res = run_bass_kernel_spmd(nc, in_maps, list(range(N_CORES)), trace=TRACE)
    if TRACE:
        _CACHE["last_exec_time_ns"] = res.exec_time_ns
        _CACHE["last_profile_json"] = res.profile_json
    return np.concatenate([res.results[c]["o"] for c in range(N_CORES)], axis=0)
